# revision 8
# baseline (speedup 1.0000x reference)
"""Deformable cross-attention Bass/Tile kernel for Trainium2 (8 NeuronCores).

Sharding: core = batch * 2 + head_group. Each core handles one batch's
queries against 4 of the 8 heads (128 of 256 value channels) and computes a
partial output through its W_out row-block; the host sums the two partials
per batch and adds b_out.

Device-side pipeline per core:
  1. value = memory @ W_val_hg + b_val_hg              (PE, fp32)
  2. 4-corner overlap tables in DRAM: tbl[h][j] = [V(j), V(j+1), V(j+w_l),
     V(j+w_l+1)] (512B rows) built with strided SBUF->DRAM DMAs
  3. stats = query @ [W_off_hg | W_attn_hg] + bias     (PE)
  4. coords/hat-weights/indices/softmax/coefs          (DVE + ACT)
  5. dma_gather of 3072 rows per (q-tile, head)        (SWDGE)
  6. combine: out = sum_j coef_j * G_j                 (DVE mult + reduce)
  7. out_proj: transpose + matmul with W_out rows      (PE)
"""

import sys

sys.path.insert(0, "/opt/trn_rl_repo")

import numpy as np

import concourse.bass as bass
import concourse.mybir as mybir
import concourse.tile as tile
from concourse import bacc
from concourse.masks import make_identity

F32 = mybir.dt.float32
I16 = mybir.dt.int16

# Problem constants (hardcoded per contest contract)
NH, L, P, HD = 8, 3, 8, 32
C = NH * HD  # 256
B, NQ = 4, 2048
LEVELS = [(100, 100), (50, 50), (25, 25)]
LSI = [0, 10000, 12500]
S = 13125
NHG = 4  # heads per core
LP = L * P  # 24


class Cfg:
    def __init__(self, levels=LEVELS, lsi=None, nq=NQ, hd=HD, nhg=NHG, l=L, p=P):
        self.levels = levels
        self.lsi = lsi if lsi is not None else list(
            np.cumsum([0] + [h * w for h, w in levels[:-1]]))
        self.S = sum(h * w for h, w in levels)
        self.nq = nq
        self.hd = hd
        self.nhg = nhg
        self.L = l
        self.P = p
        self.LP = l * p
        self.chg = nhg * hd  # channels per head group (128)
        self.n_tiles = nq // 128
        self.s_off = self.nhg * self.L * self.P * 2  # offset stat cols (192)
        self.s_attn = self.nhg * self.LP  # attn stat cols (96)
        self.s_tot = self.s_off + self.s_attn  # 288


def emit(nc, cfg, io):
    """Emit the tile program. io: dict of bass.AP DRAM handles."""
    assert cfg.chg == 128 and cfg.hd == 32
    SP = cfg.S + 1  # padded table rows
    qT, memT, ref = io["qT"], io["memT"], io["ref"]
    Wv, bv, Ws, bs, Wo, out = io["Wv"], io["bv"], io["Ws"], io["bs"], io["Wo"], io["out"]

    with tile.TileContext(nc) as tc:
        with (
            tc.tile_pool(name="const", bufs=1) as const_p,
            tc.tile_pool(name="dram", bufs=1, space="DRAM") as dram_p,
            tc.tile_pool(name="stage", bufs=2, space="DRAM") as stage_p,
            tc.tile_pool(name="vps", bufs=2, space="PSUM") as vps_p,
            tc.tile_pool(name="sps", bufs=2, space="PSUM") as sps_p,
            tc.tile_pool(name="tps", bufs=2, space="PSUM") as tps_p,
            tc.tile_pool(name="ops", bufs=2, space="PSUM") as ops_p,
            tc.tile_pool(name="vchunk", bufs=3) as vchunk_p,
            tc.tile_pool(name="stats", bufs=2) as stats_p,
            tc.tile_pool(name="gath", bufs=2) as gath_p,
            tc.tile_pool(name="wrk", bufs=2) as wrk_p,
            tc.tile_pool(name="sm", bufs=2) as sm_p,
        ):
            # ---- constants ----
            ident = const_p.tile([128, 128], F32)
            make_identity(nc, ident[:])

            Wv_sb = const_p.tile([128, 2, 128], F32, tag="wv")
            nc.sync.dma_start(out=Wv_sb[:], in_=Wv[:, :].rearrange("(a k) n -> k a n", a=2))
            bv_sb = const_p.tile([128, 128], F32, tag="bv")
            nc.sync.dma_start(out=bv_sb[:], in_=bv[:, :].to_broadcast([128, 128]))
            Ws_sb = const_p.tile([128, 2, cfg.s_tot], F32, tag="ws")
            nc.sync.dma_start(out=Ws_sb[:], in_=Ws[:, :].rearrange("(a k) n -> k a n", a=2))
            bs_sb = const_p.tile([128, cfg.s_tot], F32, tag="bs")
            nc.sync.dma_start(out=bs_sb[:], in_=bs[:, :].to_broadcast([128, cfg.s_tot]))
            Wo_sb = const_p.tile([128, 256], F32, tag="wo")
            nc.sync.dma_start(out=Wo_sb[:], in_=Wo[:, :])

            # per-level constant rows: value w_l-2 / w_l / lsi_l (same for x,y)
            wb2 = const_p.tile([128, cfg.L, 2], F32, tag="wb2")
            for li, (h, w) in enumerate(cfg.levels):
                nc.vector.memset(wb2[:, li, 0:1], float(w - 2))
                nc.vector.memset(wb2[:, li, 1:2], float(h - 2))

            # ---- gather tables ----
            tbl = dram_p.tile([cfg.nhg * SP, 4 * cfg.hd], F32, tag="tbl")
            tbl3 = tbl[:].rearrange("(h j) d -> h j d", h=cfg.nhg)

            # zero the tail rows of each level (their upper corners fall past
            # the level end and are never written by the corner copies)
            zero_sb = const_p.tile([128, 4 * cfg.hd], F32, tag="zero")
            nc.vector.memset(zero_sb[:], 0.0)
            for li, (h, w) in enumerate(cfg.levels):
                lsi, hw = cfg.lsi[li], h * w
                jr0 = max(lsi, lsi + hw - w - 2)
                jr1 = lsi + hw + (1 if li == len(cfg.levels) - 1 else 0)
                cnt = jr1 - jr0
                assert cnt <= 128
                for hh in range(cfg.nhg):
                    nc.sync.dma_start(
                        out=tbl[hh * SP + jr0:hh * SP + jr1, :],
                        in_=zero_sb[:cnt, :])

            # ---- value projection + table build ----
            with tc.tile_pool(name="mem", bufs=1) as mem_p:
                memT_sb = mem_p.tile([128, 2, cfg.S], F32)
                nc.sync.dma_start(
                    out=memT_sb[:], in_=memT[:, :].rearrange("(a k) s -> k a s", a=2))

                for li, (h, w) in enumerate(cfg.levels):
                    lsi, hw = cfg.lsi[li], h * w
                    deltas = [0, 1, w, w + 1]
                    s0 = lsi
                    while s0 < lsi + hw:
                        n = min(128, lsi + hw - s0)
                        vps = vps_p.tile([128, 128], F32, tag="vps")
                        for k in range(2):
                            nc.tensor.matmul(
                                vps[:n, :],
                                lhsT=memT_sb[:, k, s0:s0 + n],
                                rhs=Wv_sb[:, k, :],
                                start=(k == 0), stop=(k == 1),
                            )
                        vt = vchunk_p.tile([128, 128], F32, tag="vt")
                        nc.vector.tensor_tensor(
                            out=vt[:n, :], in0=vps[:n, :],
                            in1=bv_sb[:n, :],
                            op=mybir.AluOpType.add)
                        # write 4 corner-shifted copies into the table
                        for c, d in enumerate(deltas):
                            j0 = max(lsi, s0 - d)
                            j1 = (s0 + n) - d
                            if j1 <= j0:
                                continue
                            cnt = j1 - j0
                            po = (j0 + d) - s0  # src partition offset
                            dst = tbl3[:, j0:j0 + cnt, c * cfg.hd:(c + 1) * cfg.hd]
                            dst = dst.transpose([1, 0, 2])  # (j, h, d)
                            src = vt[po:po + cnt, :].rearrange(
                                "j (h d) -> j h d", h=cfg.nhg)
                            nc.sync.dma_start(out=dst, in_=src)
                        s0 += n

            # ---- load qT / ref ----
            qT_sb = const_p.tile([128, 2, cfg.nq], F32, tag="qt")
            nc.sync.dma_start(out=qT_sb[:], in_=qT[:, :].rearrange("(a k) q -> k a q", a=2))

            # ---- per q-tile main loop ----
            for t in range(cfg.n_tiles):
                q0 = t * 128
                # stats matmul
                sps = sps_p.tile([128, cfg.s_tot], F32, tag="sps")
                for k in range(2):
                    nc.tensor.matmul(
                        sps[:, :], lhsT=qT_sb[:, k, q0:q0 + 128], rhs=Ws_sb[:, k, :],
                        start=(k == 0), stop=(k == 1))
                st = stats_p.tile([128, cfg.s_tot], F32, tag="st")
                nc.vector.tensor_tensor(
                    out=st[:, :], in0=sps[:, :],
                    in1=bs_sb[:, :],
                    op=mybir.AluOpType.add)

                ref_sb = sm_p.tile([128, 2], F32, tag="ref")
                nc.sync.dma_start(out=ref_sb[:], in_=ref[q0:q0 + 128, :])

                # blp[q, l, xy] = ref*dim - 0.5
                blp = sm_p.tile([128, cfg.L, 2], F32, tag="blp")
                for li, (h, w) in enumerate(cfg.levels):
                    nc.vector.tensor_scalar(
                        out=blp[:, li, :], in0=ref_sb[:, :],
                        scalar1=float(w), scalar2=-0.5,
                        op0=mybir.AluOpType.mult, op1=mybir.AluOpType.add)

                n_off = cfg.s_off  # 192
                offs_v = st[:, 0:n_off].rearrange(
                    "p (h l pt c) -> p h l pt c", h=cfg.nhg, l=cfg.L, pt=cfg.P)
                xy = wrk_p.tile([128, n_off], F32, tag="xy")
                xy_v = xy[:].rearrange("p (h l pt c) -> p h l pt c",
                                       h=cfg.nhg, l=cfg.L, pt=cfg.P)
                for li in range(cfg.L):
                    blp_b = blp[:, li, :].unsqueeze(1).unsqueeze(2).to_broadcast(
                        [128, cfg.nhg, cfg.P, 2])
                    nc.vector.tensor_tensor(
                        out=xy_v[:, :, li], in0=offs_v[:, :, li],
                        in1=blp_b, op=mybir.AluOpType.add)

                # floor(x) = round(x-0.5) via the 1.5*2^23 magic constant
                # (off-by-one only at exact integers, where the affected
                # corner has hat-weight 0)
                MAGIC = 12582912.0  # 1.5 * 2**23
                xy0 = wrk_p.tile([128, n_off], F32, tag="xy0")
                nc.vector.tensor_scalar(
                    out=xy0[:, :], in0=xy[:, :], scalar1=-0.5, scalar2=MAGIC,
                    op0=mybir.AluOpType.add, op1=mybir.AluOpType.add)
                nc.vector.tensor_scalar(
                    out=xy0[:, :], in0=xy0[:, :], scalar1=-MAGIC, scalar2=None,
                    op0=mybir.AluOpType.add)
                # clamp to [0, dim-2]
                nc.vector.tensor_scalar(
                    out=xy0[:, :], in0=xy0[:, :], scalar1=0.0, scalar2=None,
                    op0=mybir.AluOpType.max)
                xy0_v = xy0[:].rearrange("p (h l pt c) -> p h l pt c",
                                         h=cfg.nhg, l=cfg.L, pt=cfg.P)
                for li in range(cfg.L):
                    wb2_b = wb2[:, li, :].unsqueeze(1).unsqueeze(2).to_broadcast(
                        [128, cfg.nhg, cfg.P, 2])
                    nc.vector.tensor_tensor(
                        out=xy0_v[:, :, li], in0=xy0_v[:, :, li], in1=wb2_b,
                        op=mybir.AluOpType.min)

                # d0 = xy - xy0 ; hat weights
                d0 = wrk_p.tile([128, n_off], F32, tag="d0")
                nc.vector.tensor_tensor(out=d0[:, :], in0=xy[:, :], in1=xy0[:, :],
                                        op=mybir.AluOpType.subtract)
                w0 = wrk_p.tile([128, n_off], F32, tag="w0")
                w1 = wrk_p.tile([128, n_off], F32, tag="w1")
                a0 = wrk_p.tile([128, n_off], F32, tag="a0")
                a1 = wrk_p.tile([128, n_off], F32, tag="a1")
                # w0 = max(0, 1-|d0|) = max(0, min(1-d0, 1+d0))
                nc.vector.tensor_scalar(
                    out=a0[:, :], in0=d0[:, :], scalar1=-1.0, scalar2=1.0,
                    op0=mybir.AluOpType.mult, op1=mybir.AluOpType.add)
                nc.vector.tensor_scalar(
                    out=a1[:, :], in0=d0[:, :], scalar1=1.0, scalar2=None,
                    op0=mybir.AluOpType.add)
                nc.vector.tensor_tensor(out=w0[:, :], in0=a0[:, :], in1=a1[:, :],
                                        op=mybir.AluOpType.min)
                nc.vector.tensor_scalar(
                    out=w0[:, :], in0=w0[:, :], scalar1=0.0, scalar2=None,
                    op0=mybir.AluOpType.max)
                # w1 = max(0, 1-|d0-1|) = max(0, min(2-d0, d0))
                nc.vector.tensor_scalar(
                    out=a1[:, :], in0=a0[:, :], scalar1=1.0, scalar2=None,
                    op0=mybir.AluOpType.add)
                nc.vector.tensor_tensor(out=w1[:, :], in0=a1[:, :], in1=d0[:, :],
                                        op=mybir.AluOpType.min)
                nc.vector.tensor_scalar(
                    out=w1[:, :], in0=w1[:, :], scalar1=0.0, scalar2=None,
                    op0=mybir.AluOpType.max)

                # indices: jy = y0*w + lsi ; idx = jy + x0
                n_y = cfg.nhg * cfg.LP  # 96
                jy = wrk_p.tile([128, n_y], F32, tag="jy")
                jy_v = jy[:].rearrange("p (h l pt) -> p h l pt",
                                       h=cfg.nhg, l=cfg.L)
                xy0_5 = xy0[:].rearrange("p (h l pt c) -> p h l pt c",
                                         h=cfg.nhg, l=cfg.L, pt=cfg.P)
                for li, (h, w) in enumerate(cfg.levels):
                    nc.vector.tensor_scalar(
                        out=jy_v[:, :, li, :],
                        in0=xy0_5[:, :, li, :, 1],
                        scalar1=float(w), scalar2=float(cfg.lsi[li]),
                        op0=mybir.AluOpType.mult, op1=mybir.AluOpType.add)
                idxf = wrk_p.tile([128, n_y], F32, tag="idxf")
                nc.vector.tensor_tensor(
                    out=idxf[:, :], in0=jy[:, :], in1=xy0_5[:, :, :, :, 0],
                    op=mybir.AluOpType.add)
                # wrap to [16, (h,lp,j)] + 8x replicate for dma_gather:
                # PE-transpose idxf to [(h,lp), q], swizzle q=(j,p)->(p,j)
                # during the int16 convert so the DRAM wrap-read has
                # j-contiguous 16B runs.
                psT = tps_p.tile([n_y, 128], F32, tag="tps")
                nc.tensor.transpose(psT[:, :], idxf[:, :], ident[:])
                idxTs = wrk_p.tile([n_y, 128], I16, tag="idxTs")
                nc.vector.tensor_copy(
                    out=idxTs[:].rearrange("c (p j) -> c p j", p=16),
                    in_=psT[:].rearrange("c (j p) -> c p j", j=8))
                stg = stage_p.tile([n_y, 128], I16, tag="stg")
                nc.sync.dma_start(out=stg[:], in_=idxTs[:, :])
                idxw = wrk_p.tile([128, cfg.nhg, cfg.LP, 8], I16, tag="idxw")
                stg_v = stg[:].rearrange("(h l) (p j) -> p h l j",
                                         h=cfg.nhg, p=16)
                for r in range(8):
                    nc.sync.dma_start(out=idxw[16 * r:16 * (r + 1)], in_=stg_v)

                # softmax over (l,p) per head
                sa = st[:, n_off:n_off + n_y].rearrange("p (h g) -> p h g", h=cfg.nhg)
                rmax = sm_p.tile([128, cfg.nhg], F32, tag="rmax")
                nc.vector.tensor_reduce(
                    out=rmax[:, :], in_=sa, axis=mybir.AxisListType.X,
                    op=mybir.AluOpType.max)
                esub = sm_p.tile([128, n_y], F32, tag="esub")
                nc.vector.tensor_tensor(
                    out=esub[:].rearrange("p (h g) -> p h g", h=cfg.nhg), in0=sa,
                    in1=rmax[:].unsqueeze(2).to_broadcast([128, cfg.nhg, cfg.LP]),
                    op=mybir.AluOpType.subtract)
                ex = sm_p.tile([128, n_y], F32, tag="ex")
                nc.scalar.activation(ex[:, :], esub[:, :],
                                     mybir.ActivationFunctionType.Exp)
                ssum = sm_p.tile([128, cfg.nhg], F32, tag="ssum")
                nc.vector.tensor_reduce(
                    out=ssum[:, :],
                    in_=ex[:].rearrange("p (h g) -> p h g", h=cfg.nhg),
                    axis=mybir.AxisListType.X, op=mybir.AluOpType.add)
                rcp = sm_p.tile([128, cfg.nhg], F32, tag="rcp")
                nc.vector.reciprocal(rcp[:, :], ssum[:, :])
                attn = sm_p.tile([128, n_y], F32, tag="attn")
                nc.vector.tensor_tensor(
                    out=attn[:].rearrange("p (h g) -> p h g", h=cfg.nhg),
                    in0=ex[:].rearrange("p (h g) -> p h g", h=cfg.nhg),
                    in1=rcp[:].unsqueeze(2).to_broadcast([128, cfg.nhg, cfg.LP]),
                    op=mybir.AluOpType.mult)

                # coefs C[q, (h,lp), c] c=(row,pos)
                w0x = w0[:].rearrange("p (g c) -> p c g", c=2)[:, 0, :]
                w0y = w0[:].rearrange("p (g c) -> p c g", c=2)[:, 1, :]
                w1x = w1[:].rearrange("p (g c) -> p c g", c=2)[:, 0, :]
                w1y = w1[:].rearrange("p (g c) -> p c g", c=2)[:, 1, :]
                ty0 = sm_p.tile([128, n_y], F32, tag="ty0")
                ty1 = sm_p.tile([128, n_y], F32, tag="ty1")
                nc.vector.tensor_tensor(out=ty0[:, :], in0=attn[:, :], in1=w0y,
                                        op=mybir.AluOpType.mult)
                nc.vector.tensor_tensor(out=ty1[:, :], in0=attn[:, :], in1=w1y,
                                        op=mybir.AluOpType.mult)
                coef = sm_p.tile([128, n_y, 4], F32, tag="coef")
                cv = coef[:].rearrange("p g c -> p c g")
                nc.vector.tensor_tensor(out=cv[:, 0, :], in0=ty0[:, :], in1=w0x,
                                        op=mybir.AluOpType.mult)
                nc.vector.tensor_tensor(out=cv[:, 1, :], in0=ty0[:, :], in1=w1x,
                                        op=mybir.AluOpType.mult)
                nc.vector.tensor_tensor(out=cv[:, 2, :], in0=ty1[:, :], in1=w0x,
                                        op=mybir.AluOpType.mult)
                nc.vector.tensor_tensor(out=cv[:, 3, :], in0=ty1[:, :], in1=w1x,
                                        op=mybir.AluOpType.mult)

                # gather + combine per head
                vout = gath_p.tile([128, 128], F32, tag="vout")
                for h in range(cfg.nhg):
                    g = gath_p.tile([128, cfg.LP, 4 * cfg.hd], F32, tag="g")
                    nc.gpsimd.dma_gather(
                        out_ap=g[:],
                        in_ap=tbl[h * SP:(h + 1) * SP, :],
                        idxs_ap=idxw[:, h, :, :],
                        num_idxs=cfg.LP * 128,
                        num_idxs_reg=cfg.LP * 128,
                        elem_size=4 * cfg.hd,
                        single_packet=False,
                    )
                    wt = gath_p.tile([128, cfg.LP, 4 * cfg.hd], F32, tag="wt")
                    cb = coef[:, h * cfg.LP:(h + 1) * cfg.LP, :].unsqueeze(
                        3).to_broadcast([128, cfg.LP, 4, cfg.hd])
                    nc.vector.tensor_tensor(
                        out=wt[:].rearrange("p g (c d) -> p g c d", c=4),
                        in0=g[:].rearrange("p g (c d) -> p g c d", c=4),
                        in1=cb, op=mybir.AluOpType.mult)
                    nc.vector.tensor_reduce(
                        out=vout[:, h * cfg.hd:(h + 1) * cfg.hd],
                        in_=wt[:].rearrange("p g (c d) -> p d (g c)", c=4),
                        axis=mybir.AxisListType.X, op=mybir.AluOpType.add)

                # out_proj
                tps = tps_p.tile([128, 128], F32, tag="tps")
                nc.tensor.transpose(tps[:, :], vout[:, :], ident[:])
                voT = gath_p.tile([128, 128], F32, tag="voT")
                nc.scalar.copy(voT[:, :], tps[:, :])
                ops = ops_p.tile([128, 256], F32, tag="ops")
                nc.tensor.matmul(ops[:, :], lhsT=voT[:, :], rhs=Wo_sb[:, :],
                                 start=True, stop=True)
                osb = gath_p.tile([128, 256], F32, tag="osb")
                nc.scalar.copy(osb[:, :], ops[:, :])
                nc.sync.dma_start(out=out[q0:q0 + 128, :], in_=osb[:, :])

    nc.finalize()
    return nc


def build_program(cfg=None, num_devices=8):
    cfg = cfg or Cfg()
    nc = bacc.Bacc("TRN2", target_bir_lowering=False, debug=False,
                   num_devices=num_devices)
    io = {
        "qT": nc.dram_tensor("qT", [C, cfg.nq], F32, kind="ExternalInput"),
        "memT": nc.dram_tensor("memT", [C, cfg.S], F32, kind="ExternalInput"),
        "ref": nc.dram_tensor("ref", [cfg.nq, 2], F32, kind="ExternalInput"),
        "Wv": nc.dram_tensor("Wv", [C, 128], F32, kind="ExternalInput"),
        "bv": nc.dram_tensor("bv", [1, 128], F32, kind="ExternalInput"),
        "Ws": nc.dram_tensor("Ws", [C, cfg.s_tot], F32, kind="ExternalInput"),
        "bs": nc.dram_tensor("bs", [1, cfg.s_tot], F32, kind="ExternalInput"),
        "Wo": nc.dram_tensor("Wo", [128, 256], F32, kind="ExternalInput"),
        "out": nc.dram_tensor("out", [cfg.nq, 256], F32, kind="ExternalOutput"),
    }
    io = {k: (v.ap() if hasattr(v, "ap") else v) for k, v in io.items()}
    emit(nc, cfg, io)
    return nc


def make_in_maps(query, memory, reference_points, W_off, b_off, W_attn, b_attn,
                 W_val, b_val, W_out, b_out):
    """Host-side sharding: returns (in_maps list of 8 dicts)."""
    f = np.float32
    in_maps = []
    Wof = np.asarray(W_off, f).reshape(C, NH, L * P * 2)
    bof = np.asarray(b_off, f).reshape(NH, L * P * 2)
    Wat = np.asarray(W_attn, f).reshape(C, NH, L * P)
    bat = np.asarray(b_attn, f).reshape(NH, L * P)
    for b in range(B):
        qTb = np.ascontiguousarray(np.asarray(query[b], f).T)
        memTb = np.ascontiguousarray(np.asarray(memory[b], f).T)
        refb = np.ascontiguousarray(np.asarray(reference_points[b], f))
        for hg in range(2):
            hs = slice(hg * NHG, (hg + 1) * NHG)
            Ws_c = np.concatenate(
                [Wof[:, hs].reshape(C, -1), Wat[:, hs].reshape(C, -1)], axis=1)
            bs_c = np.concatenate(
                [bof[hs].reshape(-1), bat[hs].reshape(-1)])[None, :]
            in_maps.append({
                "qT": qTb,
                "memT": memTb,
                "ref": refb,
                "Wv": np.ascontiguousarray(
                    np.asarray(W_val, f)[:, hg * 128:(hg + 1) * 128]),
                "bv": np.asarray(b_val, f)[None, hg * 128:(hg + 1) * 128],
                "Ws": np.ascontiguousarray(Ws_c),
                "bs": np.ascontiguousarray(bs_c),
                "Wo": np.ascontiguousarray(
                    np.asarray(W_out, f)[hg * 128:(hg + 1) * 128, :]),
            })
    return in_maps


_PROG = None


def kernel(query, memory, reference_points, W_off, b_off, W_attn, b_attn,
           W_val, b_val, W_out, b_out, spatial_shapes, level_start_index):
    global _PROG
    from concourse.bass_utils import run_bass_kernel_spmd

    if _PROG is None:
        _PROG = build_program()
    in_maps = make_in_maps(query, memory, reference_points, W_off, b_off,
                           W_attn, b_attn, W_val, b_val, W_out, b_out)
    import os
    trace = os.environ.get("KERNEL_TRACE", "0") == "1"
    tkw = {}
    if trace:
        tkw = dict(trace=True, trace_cores=[int(x) for x in os.environ.get(
            "KERNEL_TRACE_CORES", "0").split(",")])
    res = run_bass_kernel_spmd(_PROG, in_maps, core_ids=list(range(8)), **tkw)
    globals()["LAST_RESULT"] = res
    out = np.zeros((B, NQ, C), np.float32)
    bo = np.asarray(b_out, np.float32)
    for b in range(B):
        out[b] = res.results[2 * b]["out"] + res.results[2 * b + 1]["out"] + bo
    return out
